# revision 24
# baseline (speedup 1.0000x reference)
"""EquivariantLayerNorm forward on 8 Trainium2 NeuronCores (Bass/Tile).

Computation (irreps 256x0e + 128x1e + 64x2e + 32x3e, dim = 1184):
  - subtract the mean of the scalar (0e) block, columns [0, 256)
  - per irrep-group mean-square normalization over 4 column groups
      bounds (0, 256, 640, 960, 1184)
  - multiply by per-column weight w[irrep_idx], add bias on scalar cols

Sharding: pure data-parallel over rows, 65536 / 8 = 8192 rows per core.
Per-column weight/bias vectors are gathered on the host (tiny) and
replicated to every core.

Engine split per [128, 1184] tile:
  DVE: bn_stats/bn_aggr for the scalar block (mean + centered variance in
       one pass), centering, reciprocal, the fused (x * rstd) * w outputs
  ACT: sum of squares for the higher-l groups (Square w/ accum),
       sqrt(var + eps)

The walrus build in this toolchain accepts only ONE sync-wait per
engine/DMA instruction ("Too many sync wait commands" in codegen
otherwise), while the Tile scheduler freely attaches several. After
tracing, `_legalize_waits` splits every multi-wait instruction by
hoisting the extra waits onto standalone EventSemaphore instructions
placed immediately before it on the same engine — program order makes
this exactly equivalent.
"""

import numpy as np

import concourse.bass as bass
import concourse.tile as tile
from concourse import mybir
from concourse.bass_utils import run_bass_kernel_spmd

N_CORES = 8
N_POINTS = 65536
DIM = 1184
ROWS_PER_CORE = N_POINTS // N_CORES  # 8192
P = 128
NTILES = ROWS_PER_CORE // P  # 64
GROUP_BOUNDS = (0, 256, 640, 960, 1184)
GROUP_COUNTS = (256, 384, 320, 224)
SD = 256  # scalar (0e) block: columns [0, 256)
EPS = 1e-5
FP32 = mybir.dt.float32

# knobs read by test.py; the grading harness just calls kernel()
TRACE = False
LAST_RESULTS = None


def _build_program(plain: bool) -> bass.Bass:
    """plain=True: weight==1 and bias==0 (checked at runtime), so the
    per-column multiply/add is skipped and centering+scaling fuse into one
    tensor_scalar per block. plain=False: fully general."""
    nc = bass.Bass()
    x = nc.dram_tensor("x", [ROWS_PER_CORE, DIM], FP32, kind="ExternalInput")
    if not plain:
        w = nc.dram_tensor("w_full", [DIM], FP32, kind="ExternalInput")
        b = nc.dram_tensor("b_scalar", [SD], FP32, kind="ExternalInput")
    out = nc.dram_tensor("out", [ROWS_PER_CORE, DIM], FP32, kind="ExternalOutput")

    # macro-tiles: R row-blocks of 128 rows share one ~1.2 MB DMA transfer.
    # SBUF layout [p, (n d)]: free position n*DIM+d holds row i*R*P + n*P + p,
    # column d. Built as a raw 3-level AP (einops can't express this grouping).
    R = 4
    nmacro = NTILES // R

    def macro_ap(t, i):
        ap = t[:]
        return bass.AP(
            tensor=ap.tensor,
            offset=ap.offset + i * R * P * DIM,
            ap=[[DIM, P], [P * DIM, R], [1, DIM]],
        )

    x_t = [macro_ap(x, i) for i in range(nmacro)]
    out_t = [macro_ap(out, i) for i in range(nmacro)]

    def bcast(ap):
        # replicate a 1-D DRAM vector across all 128 partitions
        return bass.AP(tensor=ap.tensor, offset=ap.offset, ap=[[0, P]] + list(ap.ap))

    AF = mybir.ActivationFunctionType
    OP = mybir.AluOpType

    with tile.TileContext(nc) as tc:
        with (
            tc.tile_pool(name="consts", bufs=1) as consts,
            tc.tile_pool(name="xin", bufs=3) as xin,
            tc.tile_pool(name="xout", bufs=3) as xout,
            tc.tile_pool(name="sq", bufs=3) as sqp,
            tc.tile_pool(name="xcs", bufs=6) as xcsp,
            tc.tile_pool(name="small", bufs=8) as small,
        ):
            if not plain:
                wb = consts.tile([P, DIM], FP32)
                nc.gpsimd.dma_start(out=wb, in_=bcast(w[:]))
                bb = consts.tile([P, SD], FP32)
                nc.gpsimd.dma_start(out=bb, in_=bcast(b[:]))
            crec = consts.tile([P, 4], FP32)
            for g in range(4):
                nc.vector.memset(crec[:, g : g + 1], 1.0 / GROUP_COUNTS[g])
            eps_t = consts.tile([P, 1], FP32)
            nc.vector.memset(eps_t, EPS)

            for i in range(nmacro):
                xt = xin.tile([P, R * DIM], FP32)
                nc.sync.dma_start(out=xt, in_=x_t[i])

                ot = xout.tile([P, R * DIM], FP32)
                for j in range(R):
                    o = j * DIM
                    # scalar-block mean + centered variance in one DVE pass
                    stats = small.tile([P, 6], FP32)
                    nc.vector.bn_stats(out=stats, in_=xt[:, o : o + SD])
                    mv = small.tile([P, 2], FP32)
                    nc.vector.bn_aggr(out=mv, in_=stats)
                    if not plain:
                        # centered scalar block
                        xcs = xcsp.tile([P, SD], FP32)
                        nc.vector.tensor_scalar(
                            out=xcs, in0=xt[:, o : o + SD], scalar1=mv[:, 0:1],
                            scalar2=None, op0=OP.subtract,
                        )
                    # higher-l groups: sum of squares (ACT square w/ accumulate)
                    sqs = small.tile([P, 4], FP32)
                    sq = sqp.tile([P, DIM], FP32)
                    for g in range(1, 4):
                        s, e = GROUP_BOUNDS[g], GROUP_BOUNDS[g + 1]
                        nc.scalar.activation(
                            out=sq[:, s:e], in_=xt[:, o + s : o + e], func=AF.Square,
                            accum_out=sqs[:, g : g + 1],
                        )
                    # var4 = [var0, sqsum_g / count_g ...]
                    var4 = small.tile([P, 4], FP32)
                    nc.vector.tensor_copy(var4[:, 0:1], mv[:, 1:2])
                    nc.vector.tensor_mul(var4[:, 1:4], sqs[:, 1:4], crec[:, 1:4])
                    std4 = small.tile([P, 4], FP32)
                    nc.scalar.activation(out=std4, in_=var4, func=AF.Sqrt, bias=eps_t)
                    rstd = small.tile([P, 4], FP32)
                    nc.vector.reciprocal(rstd, std4)

                    if plain:
                        # out0 = (x - mean) * rstd0; out_g = x * rstd_g
                        nc.vector.tensor_scalar(
                            out=ot[:, o : o + SD], in0=xt[:, o : o + SD],
                            scalar1=mv[:, 0:1], scalar2=rstd[:, 0:1],
                            op0=OP.subtract, op1=OP.mult,
                        )
                        for g in range(1, 4):
                            s, e = GROUP_BOUNDS[g], GROUP_BOUNDS[g + 1]
                            nc.vector.tensor_scalar_mul(
                                ot[:, o + s : o + e], xt[:, o + s : o + e],
                                rstd[:, g : g + 1],
                            )
                    else:
                        # scalar block: ((x - mean) * w) * rstd0 + bias
                        t0 = xcsp.tile([P, SD], FP32)
                        nc.vector.tensor_mul(t0, xcs, wb[:, 0:SD])
                        nc.vector.scalar_tensor_tensor(
                            out=ot[:, o : o + SD], in0=t0, scalar=rstd[:, 0:1],
                            in1=bb, op0=OP.mult, op1=OP.add,
                        )
                        # higher-l groups: (x * rstd_g) * w
                        for g in range(1, 4):
                            s, e = GROUP_BOUNDS[g], GROUP_BOUNDS[g + 1]
                            nc.vector.scalar_tensor_tensor(
                                out=ot[:, o + s : o + e], in0=xt[:, o + s : o + e],
                                scalar=rstd[:, g : g + 1],
                                in1=wb[:, s:e], op0=OP.mult, op1=OP.mult,
                            )
                # store on the (otherwise idle) SWDGE queue so loads and
                # stores don't share one HWDGE FIFO
                nc.gpsimd.dma_start(out=out_t[i], in_=ot)

    _legalize_waits(nc)
    return nc


def _legalize_waits(nc: bass.Bass) -> int:
    """Split multi-wait instructions for walrus builds whose per-instruction
    sync encoding has a single wait slot.

    Every wait beyond the first is moved onto its own EventSemaphore
    instruction inserted immediately before the owning instruction on the
    same engine; the sequencer blocks there in program order, so the
    semantics are identical to the multi-wait original.
    """
    n = 0
    for blk in nc.m.functions[0].blocks:
        new_list = []
        changed = False
        for inst in blk.instructions:
            si = inst.sync_info
            if si is not None and len(si.on_wait) > 1:
                waits = list(si.on_wait)
                for w in waits[:-1]:
                    new_list.append(
                        mybir.InstEventSemaphore(
                            name=f"wsplit_{n}",
                            engine=inst.engine,
                            ins=[],
                            outs=[],
                            sync_info=mybir.SyncInfo(on_wait=[w], on_update=[]),
                        )
                    )
                    n += 1
                inst.sync_info = mybir.SyncInfo(
                    on_wait=[waits[-1]], on_update=list(si.on_update)
                )
                changed = True
            new_list.append(inst)
        if changed:
            blk.instructions = new_list
    return n


_PROGRAMS = {}


def _get_program(plain: bool) -> bass.Bass:
    if plain not in _PROGRAMS:
        _PROGRAMS[plain] = _build_program(plain)
    return _PROGRAMS[plain]


def kernel(x, weight, bias, group_idx, irrep_idx, scalar_indices, scalar_group):
    global LAST_RESULTS
    x = np.ascontiguousarray(np.asarray(x, dtype=np.float32))
    assert x.shape == (N_POINTS, DIM), x.shape
    # host-side gathers of the tiny per-column vectors
    w_full = np.ascontiguousarray(
        np.asarray(weight, np.float32)[np.asarray(irrep_idx)]
    )
    bias_full = np.zeros((DIM,), np.float32)
    bias_full[np.asarray(scalar_indices)] = np.asarray(bias, np.float32)
    b_scalar = np.ascontiguousarray(bias_full[:SD])

    # runtime specialization: identity weight/bias (the common case) lets the
    # device kernel skip the per-column multiply/add entirely
    plain = bool(np.all(w_full == 1.0) and np.all(b_scalar == 0.0))

    nc = _get_program(plain)
    in_maps = [
        {"x": x[c * ROWS_PER_CORE : (c + 1) * ROWS_PER_CORE]}
        if plain
        else {
            "x": x[c * ROWS_PER_CORE : (c + 1) * ROWS_PER_CORE],
            "w_full": w_full,
            "b_scalar": b_scalar,
        }
        for c in range(N_CORES)
    ]
    res = run_bass_kernel_spmd(nc, in_maps, list(range(N_CORES)), trace=TRACE)
    LAST_RESULTS = res
    return np.concatenate(
        [res.results[c]["out"] for c in range(N_CORES)], axis=0
    )


# revision 25
# speedup vs baseline: 1.0980x; 1.0980x over previous
"""EquivariantLayerNorm forward on 8 Trainium2 NeuronCores (Bass/Tile).

Computation (irreps 256x0e + 128x1e + 64x2e + 32x3e, dim = 1184):
  - subtract the mean of the scalar (0e) block, columns [0, 256)
  - per irrep-group mean-square normalization over 4 column groups
      bounds (0, 256, 640, 960, 1184)
  - multiply by per-column weight w[irrep_idx], add bias on scalar cols

Sharding: pure data-parallel over rows, 65536 / 8 = 8192 rows per core.
Per-column weight/bias vectors are gathered on the host (tiny) and
replicated to every core.

Engine split per [128, 1184] tile:
  DVE: bn_stats/bn_aggr for the scalar block (mean + centered variance in
       one pass), centering, reciprocal, the fused (x * rstd) * w outputs
  ACT: sum of squares for the higher-l groups (Square w/ accum),
       sqrt(var + eps)

The walrus build in this toolchain accepts only ONE sync-wait per
engine/DMA instruction ("Too many sync wait commands" in codegen
otherwise), while the Tile scheduler freely attaches several. After
tracing, `_legalize_waits` splits every multi-wait instruction by
hoisting the extra waits onto standalone EventSemaphore instructions
placed immediately before it on the same engine — program order makes
this exactly equivalent.
"""

import numpy as np

import concourse.bass as bass
import concourse.tile as tile
from concourse import mybir
from concourse.bass_utils import run_bass_kernel_spmd

N_CORES = 8
N_POINTS = 65536
DIM = 1184
ROWS_PER_CORE = N_POINTS // N_CORES  # 8192
P = 128
NTILES = ROWS_PER_CORE // P  # 64
GROUP_BOUNDS = (0, 256, 640, 960, 1184)
GROUP_COUNTS = (256, 384, 320, 224)
SD = 256  # scalar (0e) block: columns [0, 256)
EPS = 1e-5
FP32 = mybir.dt.float32

# knobs read by test.py; the grading harness just calls kernel()
TRACE = False
LAST_RESULTS = None


def _build_program(plain: bool) -> bass.Bass:
    """plain=True: weight==1 and bias==0 (checked at runtime), so the
    per-column multiply/add is skipped and centering+scaling fuse into one
    tensor_scalar per block. plain=False: fully general."""
    nc = bass.Bass()
    x = nc.dram_tensor("x", [ROWS_PER_CORE, DIM], FP32, kind="ExternalInput")
    if not plain:
        w = nc.dram_tensor("w_full", [DIM], FP32, kind="ExternalInput")
        b = nc.dram_tensor("b_scalar", [SD], FP32, kind="ExternalInput")
    out = nc.dram_tensor("out", [ROWS_PER_CORE, DIM], FP32, kind="ExternalOutput")

    # macro-tiles: R row-blocks of 128 rows share one ~1.2 MB DMA transfer.
    # SBUF layout [p, (n d)]: free position n*DIM+d holds row i*R*P + n*P + p,
    # column d. Built as a raw 3-level AP (einops can't express this grouping).
    R = 4
    nmacro = NTILES // R

    def macro_ap(t, i):
        ap = t[:]
        return bass.AP(
            tensor=ap.tensor,
            offset=ap.offset + i * R * P * DIM,
            ap=[[DIM, P], [P * DIM, R], [1, DIM]],
        )

    x_t = [macro_ap(x, i) for i in range(nmacro)]
    out_t = [macro_ap(out, i) for i in range(nmacro)]

    def bcast(ap):
        # replicate a 1-D DRAM vector across all 128 partitions
        return bass.AP(tensor=ap.tensor, offset=ap.offset, ap=[[0, P]] + list(ap.ap))

    AF = mybir.ActivationFunctionType
    OP = mybir.AluOpType

    with tile.TileContext(nc) as tc:
        with (
            tc.tile_pool(name="consts", bufs=1) as consts,
            tc.tile_pool(name="xin", bufs=4) as xin,
            tc.tile_pool(name="xout", bufs=4) as xout,
            tc.tile_pool(name="sq", bufs=3) as sqp,
            tc.tile_pool(name="xcs", bufs=6) as xcsp,
            tc.tile_pool(name="small", bufs=8) as small,
        ):
            if not plain:
                wb = consts.tile([P, DIM], FP32)
                nc.gpsimd.dma_start(out=wb, in_=bcast(w[:]))
                bb = consts.tile([P, SD], FP32)
                nc.gpsimd.dma_start(out=bb, in_=bcast(b[:]))
            crec = consts.tile([P, 4], FP32)
            for g in range(4):
                nc.vector.memset(crec[:, g : g + 1], 1.0 / GROUP_COUNTS[g])
            eps_t = consts.tile([P, 1], FP32)
            nc.vector.memset(eps_t, EPS)

            for i in range(nmacro):
                xt = xin.tile([P, R * DIM], FP32)
                nc.sync.dma_start(out=xt, in_=x_t[i])

                ot = xout.tile([P, R * DIM], FP32)
                for j in range(R):
                    o = j * DIM
                    # scalar-block mean + centered variance in one DVE pass
                    stats = small.tile([P, 6], FP32)
                    nc.vector.bn_stats(out=stats, in_=xt[:, o : o + SD])
                    mv = small.tile([P, 2], FP32)
                    nc.vector.bn_aggr(out=mv, in_=stats)
                    if not plain:
                        # centered scalar block
                        xcs = xcsp.tile([P, SD], FP32)
                        nc.vector.tensor_scalar(
                            out=xcs, in0=xt[:, o : o + SD], scalar1=mv[:, 0:1],
                            scalar2=None, op0=OP.subtract,
                        )
                    # higher-l groups: sum of squares (ACT square w/ accumulate)
                    sqs = small.tile([P, 4], FP32)
                    sq = sqp.tile([P, DIM], FP32)
                    for g in range(1, 4):
                        s, e = GROUP_BOUNDS[g], GROUP_BOUNDS[g + 1]
                        nc.scalar.activation(
                            out=sq[:, s:e], in_=xt[:, o + s : o + e], func=AF.Square,
                            accum_out=sqs[:, g : g + 1],
                        )
                    # var4 = [var0, sqsum_g / count_g ...]
                    var4 = small.tile([P, 4], FP32)
                    nc.vector.tensor_copy(var4[:, 0:1], mv[:, 1:2])
                    nc.vector.tensor_mul(var4[:, 1:4], sqs[:, 1:4], crec[:, 1:4])
                    std4 = small.tile([P, 4], FP32)
                    nc.scalar.activation(out=std4, in_=var4, func=AF.Sqrt, bias=eps_t)
                    rstd = small.tile([P, 4], FP32)
                    nc.vector.reciprocal(rstd, std4)

                    if plain:
                        # out0 = (x - mean) * rstd0; out_g = x * rstd_g
                        nc.vector.tensor_scalar(
                            out=ot[:, o : o + SD], in0=xt[:, o : o + SD],
                            scalar1=mv[:, 0:1], scalar2=rstd[:, 0:1],
                            op0=OP.subtract, op1=OP.mult,
                        )
                        for g in range(1, 4):
                            s, e = GROUP_BOUNDS[g], GROUP_BOUNDS[g + 1]
                            nc.vector.tensor_scalar_mul(
                                ot[:, o + s : o + e], xt[:, o + s : o + e],
                                rstd[:, g : g + 1],
                            )
                    else:
                        # scalar block: ((x - mean) * w) * rstd0 + bias
                        t0 = xcsp.tile([P, SD], FP32)
                        nc.vector.tensor_mul(t0, xcs, wb[:, 0:SD])
                        nc.vector.scalar_tensor_tensor(
                            out=ot[:, o : o + SD], in0=t0, scalar=rstd[:, 0:1],
                            in1=bb, op0=OP.mult, op1=OP.add,
                        )
                        # higher-l groups: (x * rstd_g) * w
                        for g in range(1, 4):
                            s, e = GROUP_BOUNDS[g], GROUP_BOUNDS[g + 1]
                            nc.vector.scalar_tensor_tensor(
                                out=ot[:, o + s : o + e], in0=xt[:, o + s : o + e],
                                scalar=rstd[:, g : g + 1],
                                in1=wb[:, s:e], op0=OP.mult, op1=OP.mult,
                            )
                # store on the (otherwise idle) SWDGE queue so loads and
                # stores don't share one HWDGE FIFO
                nc.gpsimd.dma_start(out=out_t[i], in_=ot)

    _legalize_waits(nc)
    return nc


def _legalize_waits(nc: bass.Bass) -> int:
    """Split multi-wait instructions for walrus builds whose per-instruction
    sync encoding has a single wait slot.

    Every wait beyond the first is moved onto its own EventSemaphore
    instruction inserted immediately before the owning instruction on the
    same engine; the sequencer blocks there in program order, so the
    semantics are identical to the multi-wait original.
    """
    n = 0
    for blk in nc.m.functions[0].blocks:
        new_list = []
        changed = False
        for inst in blk.instructions:
            si = inst.sync_info
            if si is not None and len(si.on_wait) > 1:
                waits = list(si.on_wait)
                for w in waits[:-1]:
                    new_list.append(
                        mybir.InstEventSemaphore(
                            name=f"wsplit_{n}",
                            engine=inst.engine,
                            ins=[],
                            outs=[],
                            sync_info=mybir.SyncInfo(on_wait=[w], on_update=[]),
                        )
                    )
                    n += 1
                inst.sync_info = mybir.SyncInfo(
                    on_wait=[waits[-1]], on_update=list(si.on_update)
                )
                changed = True
            new_list.append(inst)
        if changed:
            blk.instructions = new_list
    return n


_PROGRAMS = {}


def _get_program(plain: bool) -> bass.Bass:
    if plain not in _PROGRAMS:
        _PROGRAMS[plain] = _build_program(plain)
    return _PROGRAMS[plain]


def kernel(x, weight, bias, group_idx, irrep_idx, scalar_indices, scalar_group):
    global LAST_RESULTS
    x = np.ascontiguousarray(np.asarray(x, dtype=np.float32))
    assert x.shape == (N_POINTS, DIM), x.shape
    # host-side gathers of the tiny per-column vectors
    w_full = np.ascontiguousarray(
        np.asarray(weight, np.float32)[np.asarray(irrep_idx)]
    )
    bias_full = np.zeros((DIM,), np.float32)
    bias_full[np.asarray(scalar_indices)] = np.asarray(bias, np.float32)
    b_scalar = np.ascontiguousarray(bias_full[:SD])

    # runtime specialization: identity weight/bias (the common case) lets the
    # device kernel skip the per-column multiply/add entirely
    plain = bool(np.all(w_full == 1.0) and np.all(b_scalar == 0.0))

    nc = _get_program(plain)
    in_maps = [
        {"x": x[c * ROWS_PER_CORE : (c + 1) * ROWS_PER_CORE]}
        if plain
        else {
            "x": x[c * ROWS_PER_CORE : (c + 1) * ROWS_PER_CORE],
            "w_full": w_full,
            "b_scalar": b_scalar,
        }
        for c in range(N_CORES)
    ]
    res = run_bass_kernel_spmd(nc, in_maps, list(range(N_CORES)), trace=TRACE)
    LAST_RESULTS = res
    return np.concatenate(
        [res.results[c]["out"] for c in range(N_CORES)], axis=0
    )


# revision 26
# speedup vs baseline: 1.2338x; 1.1237x over previous
"""EquivariantLayerNorm forward on 8 Trainium2 NeuronCores (Bass/Tile).

Computation (irreps 256x0e + 128x1e + 64x2e + 32x3e, dim = 1184):
  - subtract the mean of the scalar (0e) block, columns [0, 256)
  - per irrep-group mean-square normalization over 4 column groups
      bounds (0, 256, 640, 960, 1184)
  - multiply by per-column weight w[irrep_idx], add bias on scalar cols

Sharding: pure data-parallel over rows, 65536 / 8 = 8192 rows per core.
Per-column weight/bias vectors are gathered on the host (tiny) and
replicated to every core.

Engine split per [128, 1184] tile:
  DVE: bn_stats/bn_aggr for the scalar block (mean + centered variance in
       one pass), centering, reciprocal, the fused (x * rstd) * w outputs
  ACT: sum of squares for the higher-l groups (Square w/ accum),
       sqrt(var + eps)

The walrus build in this toolchain accepts only ONE sync-wait per
engine/DMA instruction ("Too many sync wait commands" in codegen
otherwise), while the Tile scheduler freely attaches several. After
tracing, `_legalize_waits` splits every multi-wait instruction by
hoisting the extra waits onto standalone EventSemaphore instructions
placed immediately before it on the same engine — program order makes
this exactly equivalent.
"""

import numpy as np

import concourse.bass as bass
import concourse.tile as tile
from concourse import mybir
from concourse.bass_utils import run_bass_kernel_spmd

N_CORES = 8
N_POINTS = 65536
DIM = 1184
ROWS_PER_CORE = N_POINTS // N_CORES  # 8192
P = 128
NTILES = ROWS_PER_CORE // P  # 64
GROUP_BOUNDS = (0, 256, 640, 960, 1184)
GROUP_COUNTS = (256, 384, 320, 224)
SD = 256  # scalar (0e) block: columns [0, 256)
EPS = 1e-5
FP32 = mybir.dt.float32

# knobs read by test.py; the grading harness just calls kernel()
TRACE = False
LAST_RESULTS = None


def _build_program(plain: bool) -> bass.Bass:
    """plain=True: weight==1 and bias==0 (checked at runtime), so the
    per-column multiply/add is skipped and centering+scaling fuse into one
    tensor_scalar per block. plain=False: fully general."""
    nc = bass.Bass()
    x = nc.dram_tensor("x", [ROWS_PER_CORE, DIM], FP32, kind="ExternalInput")
    if not plain:
        w = nc.dram_tensor("w_full", [DIM], FP32, kind="ExternalInput")
        b = nc.dram_tensor("b_scalar", [SD], FP32, kind="ExternalInput")
    out = nc.dram_tensor("out", [ROWS_PER_CORE, DIM], FP32, kind="ExternalOutput")

    # macro-tiles: R row-blocks of 128 rows share one ~1.2 MB DMA transfer.
    # SBUF layout [p, (n d)]: free position n*DIM+d holds row i*R*P + n*P + p,
    # column d. Built as a raw 3-level AP (einops can't express this grouping).
    R = 4
    nmacro = NTILES // R

    def macro_ap(t, i):
        ap = t[:]
        return bass.AP(
            tensor=ap.tensor,
            offset=ap.offset + i * R * P * DIM,
            ap=[[DIM, P], [P * DIM, R], [1, DIM]],
        )

    x_t = [macro_ap(x, i) for i in range(nmacro)]
    out_t = [macro_ap(out, i) for i in range(nmacro)]

    def bcast(ap):
        # replicate a 1-D DRAM vector across all 128 partitions
        return bass.AP(tensor=ap.tensor, offset=ap.offset, ap=[[0, P]] + list(ap.ap))

    AF = mybir.ActivationFunctionType
    OP = mybir.AluOpType

    with tile.TileContext(nc) as tc:
        with (
            tc.tile_pool(name="consts", bufs=1) as consts,
            tc.tile_pool(name="xin", bufs=4) as xin,
            tc.tile_pool(name="xout", bufs=4) as xout,
            tc.tile_pool(name="sq", bufs=3) as sqp,
            tc.tile_pool(name="xcs", bufs=6) as xcsp,
            tc.tile_pool(name="small", bufs=8) as small,
        ):
            if not plain:
                wb = consts.tile([P, DIM], FP32)
                nc.gpsimd.dma_start(out=wb, in_=bcast(w[:]))
                bb = consts.tile([P, SD], FP32)
                nc.gpsimd.dma_start(out=bb, in_=bcast(b[:]))
            crec = consts.tile([P, 4], FP32)
            for g in range(4):
                nc.vector.memset(crec[:, g : g + 1], 1.0 / GROUP_COUNTS[g])
            eps_t = consts.tile([P, 1], FP32)
            nc.vector.memset(eps_t, EPS)

            for i in range(nmacro):
                xt = xin.tile([P, R * DIM], FP32)
                nc.sync.dma_start(out=xt, in_=x_t[i])

                ot = xout.tile([P, R * DIM], FP32)
                for j in range(R):
                    o = j * DIM
                    # scalar-block mean + centered variance in one DVE pass
                    stats = small.tile([P, 6], FP32)
                    nc.vector.bn_stats(out=stats, in_=xt[:, o : o + SD])
                    mv = small.tile([P, 2], FP32)
                    nc.vector.bn_aggr(out=mv, in_=stats)
                    if not plain:
                        # centered scalar block
                        xcs = xcsp.tile([P, SD], FP32)
                        nc.vector.tensor_scalar(
                            out=xcs, in0=xt[:, o : o + SD], scalar1=mv[:, 0:1],
                            scalar2=None, op0=OP.subtract,
                        )
                    # higher-l groups: sum of squares (ACT square w/ accumulate)
                    sqs = small.tile([P, 4], FP32)
                    sq = sqp.tile([P, DIM], FP32)
                    for g in range(1, 4):
                        s, e = GROUP_BOUNDS[g], GROUP_BOUNDS[g + 1]
                        nc.scalar.activation(
                            out=sq[:, s:e], in_=xt[:, o + s : o + e], func=AF.Square,
                            accum_out=sqs[:, g : g + 1],
                        )
                    # var4 = [var0, sqsum_g / count_g ...]
                    var4 = small.tile([P, 4], FP32)
                    nc.vector.tensor_copy(var4[:, 0:1], mv[:, 1:2])
                    nc.vector.tensor_mul(var4[:, 1:4], sqs[:, 1:4], crec[:, 1:4])
                    std4 = small.tile([P, 4], FP32)
                    nc.scalar.activation(out=std4, in_=var4, func=AF.Sqrt, bias=eps_t)
                    rstd = small.tile([P, 4], FP32)
                    nc.vector.reciprocal(rstd, std4)

                    if plain:
                        # out0 = (x - mean) * rstd0; out_g = x * rstd_g
                        nc.vector.tensor_scalar(
                            out=ot[:, o : o + SD], in0=xt[:, o : o + SD],
                            scalar1=mv[:, 0:1], scalar2=rstd[:, 0:1],
                            op0=OP.subtract, op1=OP.mult,
                        )
                        for g in range(1, 4):
                            s, e = GROUP_BOUNDS[g], GROUP_BOUNDS[g + 1]
                            nc.vector.tensor_scalar_mul(
                                ot[:, o + s : o + e], xt[:, o + s : o + e],
                                rstd[:, g : g + 1],
                            )
                    else:
                        # scalar block: ((x - mean) * w) * rstd0 + bias
                        t0 = xcsp.tile([P, SD], FP32)
                        nc.vector.tensor_mul(t0, xcs, wb[:, 0:SD])
                        nc.vector.scalar_tensor_tensor(
                            out=ot[:, o : o + SD], in0=t0, scalar=rstd[:, 0:1],
                            in1=bb, op0=OP.mult, op1=OP.add,
                        )
                        # higher-l groups: (x * rstd_g) * w
                        for g in range(1, 4):
                            s, e = GROUP_BOUNDS[g], GROUP_BOUNDS[g + 1]
                            nc.vector.scalar_tensor_tensor(
                                out=ot[:, o + s : o + e], in0=xt[:, o + s : o + e],
                                scalar=rstd[:, g : g + 1],
                                in1=wb[:, s:e], op0=OP.mult, op1=OP.mult,
                            )
                # store on the (otherwise idle) SWDGE queue so loads and
                # stores don't share one HWDGE FIFO
                nc.gpsimd.dma_start(out=out_t[i], in_=ot)

    _legalize_waits(nc)
    return nc


def _legalize_waits(nc: bass.Bass) -> int:
    """Split multi-wait instructions for walrus builds whose per-instruction
    sync encoding has a single wait slot.

    Every wait beyond the first is moved onto its own EventSemaphore
    instruction inserted immediately before the owning instruction on the
    same engine; the sequencer blocks there in program order, so the
    semantics are identical to the multi-wait original.
    """
    n = 0
    for blk in nc.m.functions[0].blocks:
        new_list = []
        changed = False
        for inst in blk.instructions:
            si = inst.sync_info
            if si is not None and len(si.on_wait) > 1:
                waits = list(si.on_wait)
                for w in waits[:-1]:
                    new_list.append(
                        mybir.InstEventSemaphore(
                            name=f"wsplit_{n}",
                            engine=inst.engine,
                            ins=[],
                            outs=[],
                            sync_info=mybir.SyncInfo(on_wait=[w], on_update=[]),
                        )
                    )
                    n += 1
                inst.sync_info = mybir.SyncInfo(
                    on_wait=[waits[-1]], on_update=list(si.on_update)
                )
                changed = True
            new_list.append(inst)
        if changed:
            blk.instructions = new_list
    return n


_PROGRAMS = {}


def _get_program(plain: bool) -> bass.Bass:
    if plain not in _PROGRAMS:
        _PROGRAMS[plain] = _build_program(plain)
    return _PROGRAMS[plain]


def kernel(x, weight, bias, group_idx, irrep_idx, scalar_indices, scalar_group):
    global LAST_RESULTS
    x = np.ascontiguousarray(np.asarray(x, dtype=np.float32))
    assert x.shape == (N_POINTS, DIM), x.shape
    # host-side gathers of the tiny per-column vectors
    w_full = np.ascontiguousarray(
        np.asarray(weight, np.float32)[np.asarray(irrep_idx)]
    )
    bias_full = np.zeros((DIM,), np.float32)
    bias_full[np.asarray(scalar_indices)] = np.asarray(bias, np.float32)
    b_scalar = np.ascontiguousarray(bias_full[:SD])

    # runtime specialization: identity weight/bias (the common case) lets the
    # device kernel skip the per-column multiply/add entirely
    plain = bool(np.all(w_full == 1.0) and np.all(b_scalar == 0.0))

    nc = _get_program(plain)
    in_maps = [
        {"x": x[c * ROWS_PER_CORE : (c + 1) * ROWS_PER_CORE]}
        if plain
        else {
            "x": x[c * ROWS_PER_CORE : (c + 1) * ROWS_PER_CORE],
            "w_full": w_full,
            "b_scalar": b_scalar,
        }
        for c in range(N_CORES)
    ]
    # retry armor: the device occasionally reports a transient
    # NRT_EXEC_UNIT_UNRECOVERABLE; a clean re-run recovers it
    last_exc = None
    res = None
    for _ in range(3):
        try:
            res = run_bass_kernel_spmd(nc, in_maps, list(range(N_CORES)), trace=TRACE)
            break
        except Exception as e:  # noqa: BLE001
            last_exc = e
    if res is None:
        raise last_exc
    LAST_RESULTS = res
    return np.concatenate(
        [res.results[c]["out"] for c in range(N_CORES)], axis=0
    )


# revision 27
# speedup vs baseline: 1.2403x; 1.0053x over previous
"""EquivariantLayerNorm forward on 8 Trainium2 NeuronCores (Bass/Tile).

Computation (irreps 256x0e + 128x1e + 64x2e + 32x3e, dim = 1184):
  - subtract the mean of the scalar (0e) block, columns [0, 256)
  - per irrep-group mean-square normalization over 4 column groups
      bounds (0, 256, 640, 960, 1184)
  - multiply by per-column weight w[irrep_idx], add bias on scalar cols

Sharding: pure data-parallel over rows, 65536 / 8 = 8192 rows per core.
Per-column weight/bias vectors are gathered on the host (tiny) and
replicated to every core.

Engine split per [128, 1184] tile:
  DVE: bn_stats/bn_aggr for the scalar block (mean + centered variance in
       one pass), centering, reciprocal, the fused (x * rstd) * w outputs
  ACT: sum of squares for the higher-l groups (Square w/ accum),
       sqrt(var + eps)

The walrus build in this toolchain accepts only ONE sync-wait per
engine/DMA instruction ("Too many sync wait commands" in codegen
otherwise), while the Tile scheduler freely attaches several. After
tracing, `_legalize_waits` splits every multi-wait instruction by
hoisting the extra waits onto standalone EventSemaphore instructions
placed immediately before it on the same engine — program order makes
this exactly equivalent.
"""

import numpy as np

import concourse.bass as bass
import concourse.tile as tile
from concourse import mybir
from concourse.bass_utils import run_bass_kernel_spmd

N_CORES = 8
N_POINTS = 65536
DIM = 1184
ROWS_PER_CORE = N_POINTS // N_CORES  # 8192
P = 128
NTILES = ROWS_PER_CORE // P  # 64
GROUP_BOUNDS = (0, 256, 640, 960, 1184)
GROUP_COUNTS = (256, 384, 320, 224)
SD = 256  # scalar (0e) block: columns [0, 256)
EPS = 1e-5
FP32 = mybir.dt.float32

# knobs read by test.py; the grading harness just calls kernel()
TRACE = False
LAST_RESULTS = None


def _build_program(plain: bool) -> bass.Bass:
    """plain=True: weight==1 and bias==0 (checked at runtime), so the
    per-column multiply/add is skipped and centering+scaling fuse into one
    tensor_scalar per block. plain=False: fully general."""
    nc = bass.Bass()
    x = nc.dram_tensor("x", [ROWS_PER_CORE, DIM], FP32, kind="ExternalInput")
    if not plain:
        w = nc.dram_tensor("w_full", [DIM], FP32, kind="ExternalInput")
        b = nc.dram_tensor("b_scalar", [SD], FP32, kind="ExternalInput")
    out = nc.dram_tensor("out", [ROWS_PER_CORE, DIM], FP32, kind="ExternalOutput")

    # macro-tiles: R row-blocks of 128 rows share one ~2.4 MB DMA transfer.
    # SBUF layout [p, (n d)]: free position n*DIM+d holds row i*R*P + n*P + p,
    # column d. Built as a raw 3-level AP (einops can't express this grouping).
    R = 4
    nmacro = NTILES // R

    def macro_ap(t, i):
        ap = t[:]
        return bass.AP(
            tensor=ap.tensor,
            offset=ap.offset + i * R * P * DIM,
            ap=[[DIM, P], [P * DIM, R], [1, DIM]],
        )

    x_t = [macro_ap(x, i) for i in range(nmacro)]
    out_t = [macro_ap(out, i) for i in range(nmacro)]

    def bcast(ap):
        # replicate a 1-D DRAM vector across all 128 partitions
        return bass.AP(tensor=ap.tensor, offset=ap.offset, ap=[[0, P]] + list(ap.ap))

    AF = mybir.ActivationFunctionType
    OP = mybir.AluOpType

    with tile.TileContext(nc) as tc:
        with (
            tc.tile_pool(name="consts", bufs=1) as consts,
            tc.tile_pool(name="xin", bufs=4) as xin,
            tc.tile_pool(name="xout", bufs=4) as xout,
            tc.tile_pool(name="sq", bufs=3) as sqp,
            tc.tile_pool(name="xcs", bufs=6) as xcsp,
            tc.tile_pool(name="small", bufs=8) as small,
        ):
            if not plain:
                wb = consts.tile([P, DIM], FP32)
                nc.gpsimd.dma_start(out=wb, in_=bcast(w[:]))
                bb = consts.tile([P, SD], FP32)
                nc.gpsimd.dma_start(out=bb, in_=bcast(b[:]))
            crec = consts.tile([P, 4], FP32)
            for g in range(4):
                nc.vector.memset(crec[:, g : g + 1], 1.0 / GROUP_COUNTS[g])
            eps_t = consts.tile([P, 1], FP32)
            nc.vector.memset(eps_t, EPS)

            for i in range(nmacro):
                xt = xin.tile([P, R * DIM], FP32)
                nc.sync.dma_start(out=xt, in_=x_t[i])

                ot = xout.tile([P, R * DIM], FP32)
                for j in range(R):
                    o = j * DIM
                    # scalar-block mean + centered variance in one DVE pass
                    stats = small.tile([P, 6], FP32)
                    nc.vector.bn_stats(out=stats, in_=xt[:, o : o + SD])
                    mv = small.tile([P, 2], FP32)
                    nc.vector.bn_aggr(out=mv, in_=stats)
                    if not plain:
                        # centered scalar block
                        xcs = xcsp.tile([P, SD], FP32)
                        nc.vector.tensor_scalar(
                            out=xcs, in0=xt[:, o : o + SD], scalar1=mv[:, 0:1],
                            scalar2=None, op0=OP.subtract,
                        )
                    # higher-l groups: sum of squares (ACT square w/ accumulate)
                    sqs = small.tile([P, 4], FP32)
                    sq = sqp.tile([P, DIM], FP32)
                    for g in range(1, 4):
                        s, e = GROUP_BOUNDS[g], GROUP_BOUNDS[g + 1]
                        nc.scalar.activation(
                            out=sq[:, s:e], in_=xt[:, o + s : o + e], func=AF.Square,
                            accum_out=sqs[:, g : g + 1],
                        )
                    # var4 = [var0, sqsum_g / count_g ...]
                    var4 = small.tile([P, 4], FP32)
                    nc.vector.tensor_copy(var4[:, 0:1], mv[:, 1:2])
                    nc.vector.tensor_mul(var4[:, 1:4], sqs[:, 1:4], crec[:, 1:4])
                    std4 = small.tile([P, 4], FP32)
                    nc.scalar.activation(out=std4, in_=var4, func=AF.Sqrt, bias=eps_t)
                    rstd = small.tile([P, 4], FP32)
                    nc.vector.reciprocal(rstd, std4)

                    if plain:
                        # out0 = (x - mean) * rstd0; out_g = x * rstd_g
                        nc.vector.tensor_scalar(
                            out=ot[:, o : o + SD], in0=xt[:, o : o + SD],
                            scalar1=mv[:, 0:1], scalar2=rstd[:, 0:1],
                            op0=OP.subtract, op1=OP.mult,
                        )
                        for g in range(1, 4):
                            s, e = GROUP_BOUNDS[g], GROUP_BOUNDS[g + 1]
                            nc.vector.tensor_scalar_mul(
                                ot[:, o + s : o + e], xt[:, o + s : o + e],
                                rstd[:, g : g + 1],
                            )
                    else:
                        # scalar block: ((x - mean) * w) * rstd0 + bias
                        t0 = xcsp.tile([P, SD], FP32)
                        nc.vector.tensor_mul(t0, xcs, wb[:, 0:SD])
                        nc.vector.scalar_tensor_tensor(
                            out=ot[:, o : o + SD], in0=t0, scalar=rstd[:, 0:1],
                            in1=bb, op0=OP.mult, op1=OP.add,
                        )
                        # higher-l groups: (x * rstd_g) * w
                        for g in range(1, 4):
                            s, e = GROUP_BOUNDS[g], GROUP_BOUNDS[g + 1]
                            nc.vector.scalar_tensor_tensor(
                                out=ot[:, o + s : o + e], in0=xt[:, o + s : o + e],
                                scalar=rstd[:, g : g + 1],
                                in1=wb[:, s:e], op0=OP.mult, op1=OP.mult,
                            )
                # store on the (otherwise idle) SWDGE queue so loads and
                # stores don't share one HWDGE FIFO
                nc.gpsimd.dma_start(out=out_t[i], in_=ot)

    _legalize_waits(nc)
    return nc


def _legalize_waits(nc: bass.Bass) -> int:
    """Split multi-wait instructions for walrus builds whose per-instruction
    sync encoding has a single wait slot.

    Every wait beyond the first is moved onto its own EventSemaphore
    instruction inserted immediately before the owning instruction on the
    same engine; the sequencer blocks there in program order, so the
    semantics are identical to the multi-wait original.
    """
    n = 0
    for blk in nc.m.functions[0].blocks:
        new_list = []
        changed = False
        for inst in blk.instructions:
            si = inst.sync_info
            if si is not None and len(si.on_wait) > 1:
                waits = list(si.on_wait)
                for w in waits[:-1]:
                    new_list.append(
                        mybir.InstEventSemaphore(
                            name=f"wsplit_{n}",
                            engine=inst.engine,
                            ins=[],
                            outs=[],
                            sync_info=mybir.SyncInfo(on_wait=[w], on_update=[]),
                        )
                    )
                    n += 1
                inst.sync_info = mybir.SyncInfo(
                    on_wait=[waits[-1]], on_update=list(si.on_update)
                )
                changed = True
            new_list.append(inst)
        if changed:
            blk.instructions = new_list
    return n


_PROGRAMS = {}


def _get_program(plain: bool) -> bass.Bass:
    if plain not in _PROGRAMS:
        _PROGRAMS[plain] = _build_program(plain)
    return _PROGRAMS[plain]


def kernel(x, weight, bias, group_idx, irrep_idx, scalar_indices, scalar_group):
    global LAST_RESULTS
    x = np.ascontiguousarray(np.asarray(x, dtype=np.float32))
    assert x.shape == (N_POINTS, DIM), x.shape
    # host-side gathers of the tiny per-column vectors
    w_full = np.ascontiguousarray(
        np.asarray(weight, np.float32)[np.asarray(irrep_idx)]
    )
    bias_full = np.zeros((DIM,), np.float32)
    bias_full[np.asarray(scalar_indices)] = np.asarray(bias, np.float32)
    b_scalar = np.ascontiguousarray(bias_full[:SD])

    # runtime specialization: identity weight/bias (the common case) lets the
    # device kernel skip the per-column multiply/add entirely
    plain = bool(np.all(w_full == 1.0) and np.all(b_scalar == 0.0))

    nc = _get_program(plain)
    in_maps = [
        {"x": x[c * ROWS_PER_CORE : (c + 1) * ROWS_PER_CORE]}
        if plain
        else {
            "x": x[c * ROWS_PER_CORE : (c + 1) * ROWS_PER_CORE],
            "w_full": w_full,
            "b_scalar": b_scalar,
        }
        for c in range(N_CORES)
    ]
    # retry armor: the device occasionally reports a transient
    # NRT_EXEC_UNIT_UNRECOVERABLE; a clean re-run recovers it
    last_exc = None
    res = None
    for _ in range(3):
        try:
            res = run_bass_kernel_spmd(nc, in_maps, list(range(N_CORES)), trace=TRACE)
            break
        except Exception as e:  # noqa: BLE001
            last_exc = e
    if res is None:
        raise last_exc
    LAST_RESULTS = res
    return np.concatenate(
        [res.results[c]["out"] for c in range(N_CORES)], axis=0
    )
